# revision 1
# baseline (speedup 1.0000x reference)
"""GCNN message-passing layer on 8 Trainium2 NeuronCores (Bass/Tile).

Math (per token m, all within one sentence of L=64 tokens):
    in_pot[m]  = (rep @ W_in)[head(m)] + b_in[lab(m)]
    in_gate[m] = (rep @ W_gate_in)[head(m)] + b_gate_in[lab(m)]
    self_pot   = rep @ W_self ; self_gate = rep @ W_gate_self
    w_d = sigmoid(gate_d) * msoft_d^2
    out = relu(in_pot*w_in + self_pot*w_self) * mask

Sharding: data-parallel over BNK (160 sentences / core). All gathers stay
within a sentence, so shards are independent; weights are replicated.

Device strategy per 128-token tile (2 sentences):
  - rep arrives host-pretransposed (fp16) so DIN sits on partitions.
  - One fused matmul produces [proj_in | gate_in | gate_self]; another W_self.
  - The within-tile head gather is a matmul with a host-built one-hot scatter
    matrix; the relation-bias lookup is a matmul with a one-hot label matrix
    accumulated into the same PSUM tile (skipped when b_in==0 and
    b_gate_in==1, which setup_inputs always produces - then the gate bias
    folds into the sigmoid's bias operand).
  - Gate weighting/masking runs on ACT/DVE straight out of PSUM; relu on
    GpSimd (otherwise idle). Output DMAs ride the second HWDGE ring (ACT's)
    so input and output streams don't serialize on one ring.
"""

import numpy as np

import concourse.bass as bass
import concourse.dve_ops as dve_ops
import concourse.mybir as mybir
import concourse.tile as tile
from concourse import bacc, bass_utils
from concourse.dve_spec import C0, C1, Spec, Src0, Src1, lower as dve_lower, relu as dve_relu
from concourse.dve_uop import DveOpSpec


def _register_gated_relu_op():
    """Register a fused custom-DVE op: out = relu(in0*s0 + in1*s1).

    Replaces the three stock DVE/ACT ops of the output tail (scale, fused
    multiply-add, relu) with a single Vector instruction. The microcode is
    lowered from the Spec at trace time like the stock custom ops; only the
    opcode row and sha pin need registering.
    """
    name = "GCNN_GATED_RELU_ANT"
    for op in dve_ops.OPS:
        if op.name == name:
            return op
    spec = Spec(
        body=dve_relu(Src0 * C0 + Src1 * C1),
        reference=lambda in0, in1, s0, s1, imm2: np.maximum(
            np.nan_to_num(in0.astype(np.float32) * s0 + in1 * s1,
                          nan=0.0, posinf=np.inf, neginf=-np.inf), 0.0),
    )
    row = dve_ops._CUSTOM_DVE_ROW_BASE + len(dve_ops.OPS)
    dve_ops._SUB_OPCODE_FOR_NAME[name] = row
    shas = {}
    for ver in ("v3", "v4"):
        uops = dve_lower(spec, ver=ver)
        shas[ver] = DveOpSpec(name=name, opcode=row, uops=uops, rd1_en=True).sha(ver)
    op = dve_ops.DveOp(name, spec, subdim=False, uops_sha=shas)
    dve_ops.OPS.append(op)
    dve_ops.CUSTOM_DVE_SPECS[name] = spec
    return op


GATED_RELU = _register_gated_relu_op()

BNK, L, DIN, DOUT, NREL = 1280, 64, 512, 256, 40
NCORES = 8
SPC = BNK // NCORES          # sentences per core
TOK = SPC * L                # tokens per core (10240)
TILE_T = 128                 # tokens per device tile
KC = DIN // 128              # K chunks (4)
NTILES = TOK // TILE_T       # 80
GROUP = 4                    # tiles per DMA batch

F32 = mybir.dt.float32
F16 = mybir.dt.float16
NP_MM = np.float16
AF = mybir.ActivationFunctionType
ALU = mybir.AluOpType


def build_nc(ntiles: int = NTILES, lab_bias: bool = True, gate_bias_one: bool = False):
    """Build the per-core Bass program (same program on all cores).

    lab_bias=False drops the relation-bias gather (valid when b_in is all
    zero); gate_bias_one then adds the constant 1.0 b_gate_in bias inside
    the sigmoid.
    """
    assert ntiles % GROUP == 0
    ngroups = ntiles // GROUP
    tok = ntiles * TILE_T
    nc = bacc.Bacc("TRN2", target_bir_lowering=False, debug=False)

    # --- DRAM I/O (DMA-batched by groups of GROUP tiles) ----------------
    repT_d = nc.dram_tensor("repT", [ngroups, 128, GROUP, KC, TILE_T], F16, kind="ExternalInput")
    scatH_d = nc.dram_tensor("scatH", [ngroups, TILE_T, GROUP, TILE_T], F16, kind="ExternalInput")
    if lab_bias:
        scatL_d = nc.dram_tensor("scatL", [ngroups, NREL, GROUP, TILE_T], F16, kind="ExternalInput")
        ball_d = nc.dram_tensor("ball", [NREL, DOUT + 2], F16, kind="ExternalInput")
    wa_d = nc.dram_tensor("wa", [128, KC, DOUT + 2], F16, kind="ExternalInput")
    ws_d = nc.dram_tensor("ws", [128, KC, DOUT], F16, kind="ExternalInput")
    aux_d = nc.dram_tensor("aux", [128, ntiles, 2], F32, kind="ExternalInput")
    out_d = nc.dram_tensor("out", [tok, DOUT], F32, kind="ExternalOutput")

    with tile.TileContext(nc) as tc:
        with (
            tc.tile_pool(name="const", bufs=1) as const_pool,
            tc.tile_pool(name="rep", bufs=3) as rep_pool,
            tc.tile_pool(name="scat", bufs=3) as scat_pool,
            tc.tile_pool(name="src", bufs=4) as src_pool,
            tc.tile_pool(name="small", bufs=8) as small_pool,
            tc.tile_pool(name="big", bufs=6) as big_pool,
            tc.tile_pool(name="out", bufs=3) as out_pool,
            tc.tile_pool(name="psum", bufs=3, space="PSUM") as psum_pool,
            tc.tile_pool(name="psum2", bufs=2, space="PSUM") as psum2_pool,
        ):
            # Resident constants
            wa_sb = const_pool.tile([128, KC, DOUT + 2], F16)
            nc.sync.dma_start(wa_sb[:], wa_d[:])
            ws_sb = const_pool.tile([128, KC, DOUT], F16)
            nc.sync.dma_start(ws_sb[:], ws_d[:])
            if lab_bias:
                ball_sb = const_pool.tile([NREL, DOUT + 2], F16)
                nc.sync.dma_start(ball_sb[:], ball_d[:])
            aux_sb = const_pool.tile([128, ntiles, 2], F32)
            nc.sync.dma_start(aux_sb[:], aux_d[:])

            for g in range(ngroups):
                rep_sb = rep_pool.tile([128, GROUP, KC, TILE_T], F16)
                nc.sync.dma_start(rep_sb[:], repT_d[g])
                scath_sb = scat_pool.tile([TILE_T, GROUP, TILE_T], F16, tag="scath")
                nc.sync.dma_start(scath_sb[:], scatH_d[g])
                if lab_bias:
                    scatl_sb = scat_pool.tile([NREL, GROUP, TILE_T], F16, tag="scatl")
                    nc.sync.dma_start(scatl_sb[:], scatL_d[g])
                o_sb = out_pool.tile([128, GROUP, DOUT], F32)

                for ti in range(GROUP):
                    i = g * GROUP + ti
                    # [proj_in | gate_in | gate_self] and self potential
                    psum_a = psum_pool.tile([128, DOUT + 2], F32, tag="pa")
                    psum_b = psum2_pool.tile([128, DOUT], F32, tag="pb")
                    for kc in range(KC):
                        first, last = kc == 0, kc == KC - 1
                        nc.tensor.matmul(psum_a[:], rep_sb[:, ti, kc, :], wa_sb[:, kc, :],
                                         start=first, stop=last)
                        nc.tensor.matmul(psum_b[:], rep_sb[:, ti, kc, :], ws_sb[:, kc, :],
                                         start=first, stop=last)

                    # head-gather (+ relation bias) via scatter matmuls; the
                    # last column gathers gate_self and is unused
                    src_sb = src_pool.tile([128, DOUT + 2], F16)
                    nc.vector.tensor_copy(src_sb[:], psum_a[:, 0:DOUT + 2])
                    psum_g = psum_pool.tile([128, DOUT + 2], F32, tag="pg")
                    nc.tensor.matmul(psum_g[:], scath_sb[:, ti, :], src_sb[:],
                                     start=True, stop=not lab_bias)
                    if lab_bias:
                        nc.tensor.matmul(psum_g[:], scatl_sb[:, ti, :], ball_sb[:],
                                         start=False, stop=True)

                    # gate weights: sigmoid(gate [+1 folded bias]) * msoft^2 * mask
                    # (both sigmoids land in one [128,2] tile so a single DVE
                    # mul applies the mask pair from aux)
                    w_raw = small_pool.tile([128, 2], F32, tag="w_raw")
                    nc.scalar.activation(w_raw[:, 0:1], psum_g[:, DOUT:DOUT + 1], AF.Sigmoid,
                                         bias=1.0 if gate_bias_one else 0.0)
                    nc.scalar.activation(w_raw[:, 1:2], psum_a[:, DOUT + 1:DOUT + 2], AF.Sigmoid)
                    w_f = small_pool.tile([128, 2], F32, tag="w_f")
                    nc.vector.tensor_mul(w_f[:], w_raw[:], aux_sb[:, i, :])

                    # res = relu(in_pot*w_in + self_pot*w_self): the DVE reads
                    # only one PSUM operand per instruction, so stage self_pot
                    # through SBUF (ACT), then one fused gated-relu DVE op
                    sp_sb = big_pool.tile([128, DOUT], F32, tag="sp")
                    nc.scalar.activation(sp_sb[:], psum_b[:], AF.Copy)
                    nc.vector._custom_dve(GATED_RELU, out=o_sb[:, ti, :],
                                          in0=psum_g[:, 0:DOUT], in1=sp_sb[:],
                                          s0=w_f[:, 0:1], s1=w_f[:, 1:2])

                # one batched output DMA per group on the ACT HWDGE ring
                # (inputs use the SP ring); dst iterated p-major to match src
                out_view = out_d[g * GROUP * TILE_T:(g + 1) * GROUP * TILE_T, :].rearrange(
                    "(i p) c -> p i c", p=TILE_T)
                nc.scalar.dma_start(out_view, o_sb[:])

    nc.compile()
    return nc


def prep_core_inputs(c, rep, adj_arc, adj_lab, adj_mask_in, adj_mask_loop, mask,
                     Wa, Ws, ball, ntiles: int = NTILES, lab_bias: bool = True):
    """Build the per-core in_map (host-side shard + layout prep)."""
    tok = ntiles * TILE_T
    ngroups = ntiles // GROUP
    sh = slice(c * SPC, (c + 1) * SPC)
    rep_s = np.ascontiguousarray(rep[sh]).reshape(SPC * L, DIN)[:tok]
    x = rep_s.reshape(ngroups, GROUP, TILE_T, KC, 128)      # [g, tile, t, kc, k]
    repT = np.ascontiguousarray(x.transpose(0, 4, 1, 3, 2).astype(NP_MM))  # [g, k, tile, kc, t]

    sent = adj_arc[sh, :, 0].reshape(-1)[:tok].astype(np.int64)
    head = adj_arc[sh, :, 1].reshape(-1)[:tok].astype(np.int64)
    idx_local = sent * L + head - c * SPC * L
    t_all = np.arange(tok)
    if idx_local.min() < 0 or idx_local.max() >= tok or np.any(idx_local // TILE_T != t_all // TILE_T):
        raise ValueError("head gather escapes its 128-token tile; unsupported input structure")

    scatH = np.zeros((ngroups, TILE_T, GROUP, TILE_T), NP_MM)
    scatH[t_all // (GROUP * TILE_T), idx_local % TILE_T,
          (t_all // TILE_T) % GROUP, t_all % TILE_T] = 1.0

    msq_in = (adj_mask_in[sh] ** 2 * mask[sh]).reshape(-1)[:tok].astype(np.float32)
    msq_loop = (adj_mask_loop[sh] ** 2 * mask[sh]).reshape(-1)[:tok].astype(np.float32)
    aux = np.ascontiguousarray(
        np.stack([msq_in.reshape(ntiles, TILE_T).T, msq_loop.reshape(ntiles, TILE_T).T], axis=-1)
    )  # [128, ntiles, 2]

    in_map = {"repT": repT, "scatH": scatH, "wa": Wa, "ws": Ws, "aux": aux}
    if lab_bias:
        lab = adj_lab[sh].reshape(-1)[:tok].astype(np.int64)
        scatL = np.zeros((ngroups, NREL, GROUP, TILE_T), NP_MM)
        scatL[t_all // (GROUP * TILE_T), lab, (t_all // TILE_T) % GROUP, t_all % TILE_T] = 1.0
        in_map["scatL"] = scatL
        in_map["ball"] = ball
    return in_map


def prep_shared(W_in, b_in, W_gate_in, b_gate_in, W_self, W_gate_self):
    Wa = np.concatenate([W_in, W_gate_in, W_gate_self], axis=1).astype(np.float32)
    Wa = np.ascontiguousarray(Wa.reshape(KC, 128, DOUT + 2).transpose(1, 0, 2).astype(NP_MM))
    Ws = np.ascontiguousarray(
        np.asarray(W_self, np.float32).reshape(KC, 128, DOUT).transpose(1, 0, 2).astype(NP_MM))
    ball = np.ascontiguousarray(np.concatenate(
        [b_in, b_gate_in, np.zeros((NREL, 1), np.float32)], axis=1).astype(NP_MM))
    return Wa, Ws, ball


_NC_CACHE = {}


def get_nc(lab_bias: bool, gate_bias_one: bool):
    key = (lab_bias, gate_bias_one)
    if key not in _NC_CACHE:
        _NC_CACHE[key] = build_nc(lab_bias=lab_bias, gate_bias_one=gate_bias_one)
    return _NC_CACHE[key]


def kernel(rep, adj_mask_in, adj_mask_loop, mask, W_in, b_in, W_gate_in,
           b_gate_in, W_self, W_gate_self, adj_arc_in, adj_lab_in):
    rep = np.asarray(rep, dtype=np.float32)
    b_in = np.asarray(b_in, dtype=np.float32)
    b_gate_in = np.asarray(b_gate_in, dtype=np.float32)
    # b_in == 0 makes the relation-bias gather a no-op; constant b_gate_in
    # folds into the sigmoid bias. setup_inputs always hits this path.
    lab_bias = not (np.all(b_in == 0.0) and np.all(b_gate_in == 1.0))
    Wa, Ws, ball = prep_shared(np.asarray(W_in), b_in, np.asarray(W_gate_in),
                               b_gate_in, np.asarray(W_self), np.asarray(W_gate_self))
    adj_arc = np.asarray(adj_arc_in)
    adj_lab = np.asarray(adj_lab_in)
    in_maps = [
        prep_core_inputs(c, rep, adj_arc, adj_lab, np.asarray(adj_mask_in),
                         np.asarray(adj_mask_loop), np.asarray(mask), Wa, Ws, ball,
                         lab_bias=lab_bias)
        for c in range(NCORES)
    ]

    nc = get_nc(lab_bias, gate_bias_one=not lab_bias)
    res = bass_utils.run_bass_kernel_spmd(nc, in_maps, core_ids=list(range(NCORES)))
    out = np.concatenate([r["out"].reshape(SPC, L, DOUT) for r in res.results], axis=0)
    return out



# revision 2
# speedup vs baseline: 1.2579x; 1.2579x over previous
"""GCNN message-passing layer on 8 Trainium2 NeuronCores (Bass/Tile).

Math (per token m, all within one sentence of L=64 tokens):
    in_pot[m]  = (rep @ W_in)[head(m)] + b_in[lab(m)]
    in_gate[m] = (rep @ W_gate_in)[head(m)] + b_gate_in[lab(m)]
    self_pot   = rep @ W_self ; self_gate = rep @ W_gate_self
    w_d = sigmoid(gate_d) * msoft_d^2
    out = relu(in_pot*w_in + self_pot*w_self) * mask

Sharding: data-parallel over BNK (160 sentences / core). All gathers stay
within a sentence, so shards are independent; weights are replicated.

Device strategy per 128-token tile (2 sentences):
  - The gate paths (rep @ W_gate_*, 0.2% of the FLOPs) run on the host;
    sigmoid(gate)*msoft^2*mask folds into the one-hot scatter values (w_in
    side) and into a per-token aux vector (w_self side). The device never
    computes gates, sigmoids, or masks.
  - rep arrives host-pretransposed (fp16) so DIN sits on partitions. One
    512-column moving operand [W_in | W_self] turns the two projections
    into 4 accumulating matmuls per tile (512-cycle streams fully hide
    each LDWEIGHTS).
  - The within-tile head gather is a matmul with a host-built one-hot
    scatter matrix whose nonzeros carry w_in; it is software-pipelined one
    tile behind the projections so the in-order tensor queue never waits
    on the PSUM->fp16 cast feeding it. (A relation-bias matmul joins the
    same accumulation only when b_in != 0; setup_inputs has b_in == 0.)
  - Tail per tile: ACT casts in_pot to fp16 (gather src) and copies
    self_pot to SBUF; one fused custom-DVE op emits
    relu(in_pot_gathered + w_self*self_pot) straight to fp16.
  - Outputs leave as fp16 (host upcasts), on the GpSimd HWDGE ring so the
    input stream (SP ring) never serializes against them.
"""

import numpy as np

import concourse.bass as bass
import concourse.dve_ops as dve_ops
import concourse.mybir as mybir
import concourse.tile as tile
from concourse import bacc, bass_utils
from concourse.dve_spec import C0, C1, Spec, Src0, Src1, lower as dve_lower, relu as dve_relu
from concourse.dve_uop import DveOpSpec


def _register_gated_relu_op():
    """Register a fused custom-DVE op: out = relu(in0*s0 + in1*s1)."""
    name = "GCNN_GATED_RELU_ANT"
    for op in dve_ops.OPS:
        if op.name == name:
            return op
    spec = Spec(
        body=dve_relu(Src0 * C0 + Src1 * C1),
        reference=lambda in0, in1, s0, s1, imm2: np.maximum(
            np.nan_to_num(in0.astype(np.float32) * s0 + in1 * s1,
                          nan=0.0, posinf=np.inf, neginf=-np.inf), 0.0),
    )
    row = dve_ops._CUSTOM_DVE_ROW_BASE + len(dve_ops.OPS)
    dve_ops._SUB_OPCODE_FOR_NAME[name] = row
    shas = {}
    for ver in ("v3", "v4"):
        uops = dve_lower(spec, ver=ver)
        shas[ver] = DveOpSpec(name=name, opcode=row, uops=uops, rd1_en=True).sha(ver)
    op = dve_ops.DveOp(name, spec, subdim=False, uops_sha=shas)
    dve_ops.OPS.append(op)
    dve_ops.CUSTOM_DVE_SPECS[name] = spec
    return op


GATED_RELU = _register_gated_relu_op()

BNK, L, DIN, DOUT, NREL = 1280, 64, 512, 256, 40
NCORES = 8
SPC = BNK // NCORES          # sentences per core
TOK = SPC * L                # tokens per core (10240)
TILE_T = 128                 # tokens per device tile
KC = DIN // 128              # K chunks (4)
NTILES = TOK // TILE_T       # 80
GROUP = 8                    # tiles per DMA batch

F32 = mybir.dt.float32
F16 = mybir.dt.float16
NP_MM = np.float16
AF = mybir.ActivationFunctionType


def build_nc(ntiles: int = NTILES, lab_bias: bool = False):
    """Build the per-core Bass program (same program on all cores).

    lab_bias=True adds the relation-bias matmul (needed only when b_in is
    nonzero; setup_inputs always produces b_in == 0).
    """
    assert ntiles % GROUP == 0
    ngroups = ntiles // GROUP
    tok = ntiles * TILE_T
    nc = bacc.Bacc("TRN2", target_bir_lowering=False, debug=False)

    # --- DRAM I/O (DMA-batched by groups of GROUP tiles) ----------------
    repT_d = nc.dram_tensor("repT", [ngroups, 128, GROUP, KC, TILE_T], F16, kind="ExternalInput")
    scatW_d = nc.dram_tensor("scatW", [ngroups, TILE_T, GROUP, TILE_T], F16, kind="ExternalInput")
    if lab_bias:
        scatL_d = nc.dram_tensor("scatL", [ngroups, NREL, GROUP, TILE_T], F16, kind="ExternalInput")
        ball_d = nc.dram_tensor("ball", [NREL, DOUT], F16, kind="ExternalInput")
    wcat_d = nc.dram_tensor("wcat", [128, KC, 2 * DOUT], F16, kind="ExternalInput")
    # aux[:, i] = w_self for tile i; last column is the constant 1.0
    aux_d = nc.dram_tensor("aux", [128, ntiles + 1], F32, kind="ExternalInput")
    out_d = nc.dram_tensor("out", [tok, DOUT], F16, kind="ExternalOutput")

    with tile.TileContext(nc) as tc:
        with (
            tc.tile_pool(name="const", bufs=1) as const_pool,
            tc.tile_pool(name="rep", bufs=3) as rep_pool,
            tc.tile_pool(name="scat", bufs=3) as scat_pool,
            tc.tile_pool(name="src", bufs=3) as src_pool,
            tc.tile_pool(name="sp", bufs=3) as sp_pool,
            tc.tile_pool(name="out", bufs=2) as out_pool,
            tc.tile_pool(name="psum", bufs=3, space="PSUM") as psum_pool,
            tc.tile_pool(name="psum2", bufs=3, space="PSUM") as psum2_pool,
        ):
            # Resident constants
            wcat_sb = const_pool.tile([128, KC, 2 * DOUT], F16)
            nc.sync.dma_start(wcat_sb[:], wcat_d[:])
            if lab_bias:
                ball_sb = const_pool.tile([NREL, DOUT], F16)
                nc.sync.dma_start(ball_sb[:], ball_d[:])
            aux_sb = const_pool.tile([128, ntiles + 1], F32)
            nc.sync.dma_start(aux_sb[:], aux_d[:])
            ones = aux_sb[:, ntiles:ntiles + 1]

            # Software pipeline: the gather + tail for tile i is emitted
            # during tile i+1, after that tile's projection matmuls, so the
            # tensor queue never stalls on the ACT cast.
            pend = None          # (i, ti, psum_ab, src, o_sb, scat_sb, scatl_sb)
            pend_out = None      # (g, o_sb) awaiting its batched output DMA

            def flush_tail():
                nonlocal pend, pend_out
                if pend is None:
                    return
                i, ti, psum_ab, src, o_sb, scat_sb, scatl_sb = pend
                psum_g = psum2_pool.tile([128, DOUT], F32, tag="pg")
                nc.tensor.matmul(psum_g[:], scat_sb[:, ti, :], src[:],
                                 start=True, stop=not lab_bias)
                if lab_bias:
                    nc.tensor.matmul(psum_g[:], scatl_sb[:, ti, :], ball_sb[:],
                                     start=False, stop=True)
                sp = sp_pool.tile([128, DOUT], F32)
                nc.scalar.activation(sp[:], psum_ab[:, DOUT:2 * DOUT], AF.Copy)
                nc.vector._custom_dve(GATED_RELU, out=o_sb[:, ti, :],
                                      in0=psum_g[:], in1=sp[:],
                                      s0=ones, s1=aux_sb[:, i:i + 1])
                pend = None
                if ti == GROUP - 1:
                    g, osb = pend_out
                    out_view = out_d[g * GROUP * TILE_T:(g + 1) * GROUP * TILE_T, :].rearrange(
                        "(i p) c -> p i c", p=TILE_T)
                    nc.gpsimd.dma_start(out_view, osb[:])
                    pend_out = None

            for g in range(ngroups):
                rep_sb = rep_pool.tile([128, GROUP, KC, TILE_T], F16)
                nc.sync.dma_start(rep_sb[:], repT_d[g])
                scat_sb = scat_pool.tile([TILE_T, GROUP, TILE_T], F16, tag="scath")
                nc.sync.dma_start(scat_sb[:], scatW_d[g])
                scatl_sb = None
                if lab_bias:
                    scatl_sb = scat_pool.tile([NREL, GROUP, TILE_T], F16, tag="scatl")
                    nc.sync.dma_start(scatl_sb[:], scatL_d[g])
                o_sb = out_pool.tile([128, GROUP, DOUT], F16)

                for ti in range(GROUP):
                    i = g * GROUP + ti
                    # [in_pot | self_pot] in one PSUM bank via a fused
                    # 512-column moving operand
                    psum_ab = psum_pool.tile([128, 2 * DOUT], F32, tag="pab")
                    for kc in range(KC):
                        nc.tensor.matmul(psum_ab[:], rep_sb[:, ti, kc, :], wcat_sb[:, kc, :],
                                         start=kc == 0, stop=kc == KC - 1)
                    src = src_pool.tile([128, DOUT], F16)
                    nc.scalar.activation(src[:], psum_ab[:, 0:DOUT], AF.Copy)
                    flush_tail()
                    if ti == GROUP - 1:
                        pend_out = (g, o_sb)
                    pend = (i, ti, psum_ab, src, o_sb, scat_sb, scatl_sb)
            flush_tail()

    nc.compile()
    return nc


def _sigmoid(x):
    out = np.empty_like(x, dtype=np.float32)
    pos = x >= 0
    out[pos] = 1.0 / (1.0 + np.exp(-x[pos]))
    ex = np.exp(x[~pos])
    out[~pos] = ex / (1.0 + ex)
    return out


def prep_gates(rep_flat, adj_arc, adj_lab, adj_mask_in, adj_mask_loop, mask,
               W_gate_in, b_gate_in, W_gate_self):
    """Host gate path: per-token gate weights with masks folded in."""
    idx = (adj_arc[..., 0].reshape(-1) * L + adj_arc[..., 1].reshape(-1)).astype(np.int64)
    lab = adj_lab.reshape(-1).astype(np.int64)
    g_in = (rep_flat @ np.asarray(W_gate_in, np.float32)[:, 0])[idx] \
        + np.asarray(b_gate_in, np.float32)[lab, 0]
    g_self = rep_flat @ np.asarray(W_gate_self, np.float32)[:, 0]
    m = np.asarray(mask, np.float32).reshape(-1)
    w_in = _sigmoid(g_in) * np.asarray(adj_mask_in, np.float32).reshape(-1) ** 2 * m
    w_self = _sigmoid(g_self) * np.asarray(adj_mask_loop, np.float32).reshape(-1) ** 2 * m
    return idx, lab, w_in, w_self


def prep_core_inputs(c, rep, idx, lab, w_in, w_self, wcat, ball,
                     ntiles: int = NTILES, lab_bias: bool = False):
    """Build the per-core in_map (host-side shard + layout prep)."""
    tok = ntiles * TILE_T
    ngroups = ntiles // GROUP
    lo = c * SPC * L
    rep_s = np.ascontiguousarray(rep[c * SPC:(c + 1) * SPC]).reshape(SPC * L, DIN)[:tok]
    x = rep_s.reshape(ngroups, GROUP, TILE_T, KC, 128)      # [g, tile, t, kc, k]
    repT = np.ascontiguousarray(x.transpose(0, 4, 1, 3, 2).astype(NP_MM))  # [g, k, tile, kc, t]

    idx_local = idx[lo:lo + tok] - lo
    t_all = np.arange(tok)
    if idx_local.min() < 0 or idx_local.max() >= tok or np.any(idx_local // TILE_T != t_all // TILE_T):
        raise ValueError("head gather escapes its 128-token tile; unsupported input structure")

    w_in_s = w_in[lo:lo + tok].astype(NP_MM)
    scatW = np.zeros((ngroups, TILE_T, GROUP, TILE_T), NP_MM)
    scatW[t_all // (GROUP * TILE_T), idx_local % TILE_T,
          (t_all // TILE_T) % GROUP, t_all % TILE_T] = w_in_s

    aux = np.empty((128, ntiles + 1), np.float32)
    aux[:, :ntiles] = w_self[lo:lo + tok].reshape(ntiles, TILE_T).T
    aux[:, ntiles] = 1.0

    in_map = {"repT": repT, "scatW": scatW, "wcat": wcat, "aux": aux}
    if lab_bias:
        lab_s = lab[lo:lo + tok]
        scatL = np.zeros((ngroups, NREL, GROUP, TILE_T), NP_MM)
        scatL[t_all // (GROUP * TILE_T), lab_s, (t_all // TILE_T) % GROUP, t_all % TILE_T] = w_in_s
        in_map["scatL"] = scatL
        in_map["ball"] = ball
    return in_map


def prep_shared(W_in, b_in, W_self):
    wcat = np.concatenate([np.asarray(W_in, np.float32),
                           np.asarray(W_self, np.float32)], axis=1)
    wcat = np.ascontiguousarray(
        wcat.reshape(KC, 128, 2 * DOUT).transpose(1, 0, 2).astype(NP_MM))
    ball = np.ascontiguousarray(np.asarray(b_in, np.float32).astype(NP_MM))
    return wcat, ball


_NC_CACHE = {}


def get_nc(lab_bias: bool):
    if lab_bias not in _NC_CACHE:
        _NC_CACHE[lab_bias] = build_nc(lab_bias=lab_bias)
    return _NC_CACHE[lab_bias]


def kernel(rep, adj_mask_in, adj_mask_loop, mask, W_in, b_in, W_gate_in,
           b_gate_in, W_self, W_gate_self, adj_arc_in, adj_lab_in):
    rep = np.asarray(rep, dtype=np.float32)
    b_in = np.asarray(b_in, dtype=np.float32)
    lab_bias = bool(np.any(b_in != 0.0))
    rep_flat = rep.reshape(BNK * L, DIN)
    idx, lab, w_in, w_self = prep_gates(
        rep_flat, np.asarray(adj_arc_in), np.asarray(adj_lab_in),
        adj_mask_in, adj_mask_loop, mask, W_gate_in, b_gate_in, W_gate_self)
    wcat, ball = prep_shared(W_in, b_in, W_self)
    in_maps = [
        prep_core_inputs(c, rep, idx, lab, w_in, w_self, wcat, ball, lab_bias=lab_bias)
        for c in range(NCORES)
    ]

    nc = get_nc(lab_bias)
    res = bass_utils.run_bass_kernel_spmd(nc, in_maps, core_ids=list(range(NCORES)))
    out = np.concatenate(
        [r["out"].astype(np.float32).reshape(SPC, L, DOUT) for r in res.results], axis=0)
    return out


# revision 5
# speedup vs baseline: 1.3236x; 1.0522x over previous
"""GCNN message-passing layer on 8 Trainium2 NeuronCores (Bass/Tile).

Math (per token m, all within one sentence of L=64 tokens):
    in_pot[m]  = (rep @ W_in)[head(m)] + b_in[lab(m)]
    in_gate[m] = (rep @ W_gate_in)[head(m)] + b_gate_in[lab(m)]
    self_pot   = rep @ W_self ; self_gate = rep @ W_gate_self
    w_d = sigmoid(gate_d) * msoft_d^2
    out = relu(in_pot*w_in + self_pot*w_self) * mask

Sharding: data-parallel over BNK (160 sentences / core). All gathers stay
within a sentence, so shards are independent; weights are replicated.

Device strategy per 128-token tile (2 sentences):
  - The gate paths (rep @ W_gate_*, 0.2% of the FLOPs) run on the host;
    sigmoid(gate)*msoft^2*mask folds into the one-hot scatter values (w_in
    side) and into a per-token aux vector (w_self side). The device never
    computes gates, sigmoids, or masks.
  - rep arrives host-pretransposed (fp16) so DIN sits on partitions. One
    512-column moving operand [W_in | W_self] turns the two projections
    into 4 accumulating matmuls per tile (512-cycle streams fully hide
    each LDWEIGHTS).
  - The within-tile head gather is a matmul with a host-built one-hot
    scatter matrix whose nonzeros carry w_in; it is software-pipelined one
    tile behind the projections so the in-order tensor queue never waits
    on the PSUM->fp16 cast feeding it. (A relation-bias matmul joins the
    same accumulation only when b_in != 0; setup_inputs has b_in == 0.)
  - Tail per tile: one ACT op casts the whole [in_pot|self_pot] PSUM bank
    to fp16 (gather src + self operand), then one fused custom-DVE op
    emits relu(in_pot_gathered + w_self*self_pot) straight to fp16.
  - Output stays partition-major in DRAM ([128, ntiles, dout]) so the DMA
    moves 4KB-contiguous runs; the host de-interleaves. Outputs ride the
    GpSimd HWDGE queue, inputs the SP queue.
  - ~32 throwaway matmuls run while the first DMAs land, so the PE HAM
    clock-gate is already released (2.4 GHz) when real work arrives.
"""

import numpy as np

import concourse.bass as bass
import concourse.dve_ops as dve_ops
import concourse.mybir as mybir
import concourse.tile as tile
from concourse import bacc, bass_utils
from concourse.dve_spec import C0, C1, Spec, Src0, Src1, lower as dve_lower, relu as dve_relu
from concourse.dve_uop import DveOpSpec


def _register_gated_relu_op():
    """Register a fused custom-DVE op: out = relu(in0*s0 + in1*s1)."""
    name = "GCNN_GATED_RELU_ANT"
    for op in dve_ops.OPS:
        if op.name == name:
            return op
    spec = Spec(
        body=dve_relu(Src0 * C0 + Src1 * C1),
        reference=lambda in0, in1, s0, s1, imm2: np.maximum(
            np.nan_to_num(in0.astype(np.float32) * s0 + in1 * s1,
                          nan=0.0, posinf=np.inf, neginf=-np.inf), 0.0),
    )
    row = dve_ops._CUSTOM_DVE_ROW_BASE + len(dve_ops.OPS)
    dve_ops._SUB_OPCODE_FOR_NAME[name] = row
    shas = {}
    for ver in ("v3", "v4"):
        uops = dve_lower(spec, ver=ver)
        shas[ver] = DveOpSpec(name=name, opcode=row, uops=uops, rd1_en=True).sha(ver)
    op = dve_ops.DveOp(name, spec, subdim=False, uops_sha=shas)
    dve_ops.OPS.append(op)
    dve_ops.CUSTOM_DVE_SPECS[name] = spec
    return op


GATED_RELU = _register_gated_relu_op()

BNK, L, DIN, DOUT, NREL = 1280, 64, 512, 256, 40
NCORES = 8
SPC = BNK // NCORES          # sentences per core
TOK = SPC * L                # tokens per core (10240)
TILE_T = 128                 # tokens per device tile
KC = DIN // 128              # K chunks (4)
NTILES = TOK // TILE_T       # 80
GROUP = 4                    # tiles per DMA batch
NWARM = 32                   # HAM warmup matmuls

F32 = mybir.dt.float32
F16 = mybir.dt.float16
NP_MM = np.float16
AF = mybir.ActivationFunctionType


def build_nc(ntiles: int = NTILES, lab_bias: bool = False):
    """Build the per-core Bass program (same program on all cores).

    lab_bias=True adds the relation-bias matmul (needed only when b_in is
    nonzero; setup_inputs always produces b_in == 0).
    """
    assert ntiles % GROUP == 0
    ngroups = ntiles // GROUP
    nc = bacc.Bacc("TRN2", target_bir_lowering=False, debug=False)

    # --- DRAM I/O (DMA-batched by groups of GROUP tiles) ----------------
    repT_d = nc.dram_tensor("repT", [ngroups, 128, GROUP, KC, TILE_T], F16, kind="ExternalInput")
    scatW_d = nc.dram_tensor("scatW", [ngroups, TILE_T, GROUP, TILE_T], F16, kind="ExternalInput")
    if lab_bias:
        scatL_d = nc.dram_tensor("scatL", [ngroups, NREL, GROUP, TILE_T], F16, kind="ExternalInput")
        ball_d = nc.dram_tensor("ball", [NREL, DOUT], F16, kind="ExternalInput")
    wcat_d = nc.dram_tensor("wcat", [128, KC, 2 * DOUT], F16, kind="ExternalInput")
    # aux[:, i] = w_self for tile i; last column is the constant 1.0
    aux_d = nc.dram_tensor("aux", [128, ntiles + 1], F32, kind="ExternalInput")
    # partition-major output: [p, tile, dout]; host de-interleaves
    out_d = nc.dram_tensor("out", [TILE_T, ntiles, DOUT], F16, kind="ExternalOutput")

    with tile.TileContext(nc) as tc:
        with (
            tc.tile_pool(name="const", bufs=1) as const_pool,
            tc.tile_pool(name="rep", bufs=3) as rep_pool,
            tc.tile_pool(name="scat", bufs=3) as scat_pool,
            tc.tile_pool(name="src", bufs=3) as src_pool,
            tc.tile_pool(name="out", bufs=3) as out_pool,
            tc.tile_pool(name="psum", bufs=3, space="PSUM") as psum_pool,
            tc.tile_pool(name="psum2", bufs=3, space="PSUM") as psum2_pool,
            tc.tile_pool(name="psumw", bufs=1, space="PSUM") as psumw_pool,
        ):
            # --- PE warmup: release the HAM clock gate while DMAs land ---
            wz = const_pool.tile([128, 16], F16)
            nc.gpsimd.memset(wz[:], 0.0)
            wp = psumw_pool.tile([16, 16], F32, tag="warm")
            for _ in range(NWARM):
                nc.tensor.matmul(wp[:], wz[:, 0:16], wz[:], start=True, stop=True)

            # Resident constants (issued on the input queue after group 0's
            # rep so the first projection's data has priority)
            wcat_sb = const_pool.tile([128, KC, 2 * DOUT], F16)
            aux_sb = const_pool.tile([128, ntiles + 1], F32)
            ones = aux_sb[:, ntiles:ntiles + 1]
            ball_sb = const_pool.tile([NREL, DOUT], F16) if lab_bias else None

            pend = None          # (i, ti, src, o_sb, scat_sb, scatl_sb)
            pend_out = None      # (g, o_sb) awaiting its batched output DMA

            def flush_tail():
                nonlocal pend, pend_out
                if pend is None:
                    return
                i, ti, src, o_sb, scat_sb, scatl_sb = pend
                psum_g = psum2_pool.tile([128, DOUT], F32, tag="pg")
                nc.tensor.matmul(psum_g[:], scat_sb[:, ti, :], src[:, 0:DOUT],
                                 start=True, stop=not lab_bias)
                if lab_bias:
                    nc.tensor.matmul(psum_g[:], scatl_sb[:, ti, :], ball_sb[:],
                                     start=False, stop=True)
                nc.vector._custom_dve(GATED_RELU, out=o_sb[:, ti, :],
                                      in0=psum_g[:], in1=src[:, DOUT:2 * DOUT],
                                      s0=ones, s1=aux_sb[:, i:i + 1])
                pend = None
                if ti == GROUP - 1:
                    g, osb = pend_out
                    nc.gpsimd.dma_start(out_d[:, g * GROUP:(g + 1) * GROUP, :], osb[:])
                    pend_out = None

            for g in range(ngroups):
                rep_sb = rep_pool.tile([128, GROUP, KC, TILE_T], F16)
                nc.sync.dma_start(rep_sb[:], repT_d[g])
                if g == 0:
                    nc.sync.dma_start(wcat_sb[:], wcat_d[:])
                scat_sb = scat_pool.tile([TILE_T, GROUP, TILE_T], F16, tag="scath")
                nc.sync.dma_start(scat_sb[:], scatW_d[g])
                scatl_sb = None
                if lab_bias:
                    scatl_sb = scat_pool.tile([NREL, GROUP, TILE_T], F16, tag="scatl")
                    nc.sync.dma_start(scatl_sb[:], scatL_d[g])
                if g == 0:
                    nc.sync.dma_start(aux_sb[:], aux_d[:])
                    if lab_bias:
                        nc.sync.dma_start(ball_sb[:], ball_d[:])
                o_sb = out_pool.tile([128, GROUP, DOUT], F16)

                for ti in range(GROUP):
                    i = g * GROUP + ti
                    # [in_pot | self_pot] in one PSUM bank via a fused
                    # 512-column moving operand
                    psum_ab = psum_pool.tile([128, 2 * DOUT], F32, tag="pab")
                    for kc in range(KC):
                        nc.tensor.matmul(psum_ab[:], rep_sb[:, ti, kc, :], wcat_sb[:, kc, :],
                                         start=kc == 0, stop=kc == KC - 1)
                    src = src_pool.tile([128, 2 * DOUT], F16)
                    nc.scalar.activation(src[:], psum_ab[:], AF.Copy)
                    flush_tail()
                    if ti == GROUP - 1:
                        pend_out = (g, o_sb)
                    pend = (i, ti, src, o_sb, scat_sb, scatl_sb)
            flush_tail()

    nc.compile()
    return nc


def _sigmoid(x):
    out = np.empty_like(x, dtype=np.float32)
    pos = x >= 0
    out[pos] = 1.0 / (1.0 + np.exp(-x[pos]))
    ex = np.exp(x[~pos])
    out[~pos] = ex / (1.0 + ex)
    return out


def prep_gates(rep_flat, adj_arc, adj_lab, adj_mask_in, adj_mask_loop, mask,
               W_gate_in, b_gate_in, W_gate_self):
    """Host gate path: per-token gate weights with masks folded in."""
    idx = (adj_arc[..., 0].reshape(-1) * L + adj_arc[..., 1].reshape(-1)).astype(np.int64)
    lab = adj_lab.reshape(-1).astype(np.int64)
    g_in = (rep_flat @ np.asarray(W_gate_in, np.float32)[:, 0])[idx] \
        + np.asarray(b_gate_in, np.float32)[lab, 0]
    g_self = rep_flat @ np.asarray(W_gate_self, np.float32)[:, 0]
    m = np.asarray(mask, np.float32).reshape(-1)
    w_in = _sigmoid(g_in) * np.asarray(adj_mask_in, np.float32).reshape(-1) ** 2 * m
    w_self = _sigmoid(g_self) * np.asarray(adj_mask_loop, np.float32).reshape(-1) ** 2 * m
    return idx, lab, w_in, w_self


def prep_core_inputs(c, rep, idx, lab, w_in, w_self, wcat, ball,
                     ntiles: int = NTILES, lab_bias: bool = False):
    """Build the per-core in_map (host-side shard + layout prep)."""
    tok = ntiles * TILE_T
    ngroups = ntiles // GROUP
    lo = c * SPC * L
    rep_s = np.ascontiguousarray(rep[c * SPC:(c + 1) * SPC]).reshape(SPC * L, DIN)[:tok]
    x = rep_s.reshape(ngroups, GROUP, TILE_T, KC, 128)      # [g, tile, t, kc, k]
    repT = np.ascontiguousarray(x.transpose(0, 4, 1, 3, 2).astype(NP_MM))  # [g, k, tile, kc, t]

    idx_local = idx[lo:lo + tok] - lo
    t_all = np.arange(tok)
    if idx_local.min() < 0 or idx_local.max() >= tok or np.any(idx_local // TILE_T != t_all // TILE_T):
        raise ValueError("head gather escapes its 128-token tile; unsupported input structure")

    w_in_s = w_in[lo:lo + tok].astype(NP_MM)
    scatW = np.zeros((ngroups, TILE_T, GROUP, TILE_T), NP_MM)
    scatW[t_all // (GROUP * TILE_T), idx_local % TILE_T,
          (t_all // TILE_T) % GROUP, t_all % TILE_T] = w_in_s

    aux = np.empty((128, ntiles + 1), np.float32)
    aux[:, :ntiles] = w_self[lo:lo + tok].reshape(ntiles, TILE_T).T
    aux[:, ntiles] = 1.0

    in_map = {"repT": repT, "scatW": scatW, "wcat": wcat, "aux": aux}
    if lab_bias:
        lab_s = lab[lo:lo + tok]
        scatL = np.zeros((ngroups, NREL, GROUP, TILE_T), NP_MM)
        scatL[t_all // (GROUP * TILE_T), lab_s, (t_all // TILE_T) % GROUP, t_all % TILE_T] = w_in_s
        in_map["scatL"] = scatL
        in_map["ball"] = ball
    return in_map


def prep_shared(W_in, b_in, W_self):
    wcat = np.concatenate([np.asarray(W_in, np.float32),
                           np.asarray(W_self, np.float32)], axis=1)
    wcat = np.ascontiguousarray(
        wcat.reshape(KC, 128, 2 * DOUT).transpose(1, 0, 2).astype(NP_MM))
    ball = np.ascontiguousarray(np.asarray(b_in, np.float32).astype(NP_MM))
    return wcat, ball


def unshard_out(raw):
    """[128, ntiles, DOUT] fp16 partition-major -> [SPC, L, DOUT] fp32."""
    return raw.transpose(1, 0, 2).astype(np.float32).reshape(SPC, L, DOUT)


_NC_CACHE = {}


def get_nc(lab_bias: bool):
    if lab_bias not in _NC_CACHE:
        _NC_CACHE[lab_bias] = build_nc(lab_bias=lab_bias)
    return _NC_CACHE[lab_bias]


def kernel(rep, adj_mask_in, adj_mask_loop, mask, W_in, b_in, W_gate_in,
           b_gate_in, W_self, W_gate_self, adj_arc_in, adj_lab_in):
    rep = np.asarray(rep, dtype=np.float32)
    b_in = np.asarray(b_in, dtype=np.float32)
    lab_bias = bool(np.any(b_in != 0.0))
    rep_flat = rep.reshape(BNK * L, DIN)
    idx, lab, w_in, w_self = prep_gates(
        rep_flat, np.asarray(adj_arc_in), np.asarray(adj_lab_in),
        adj_mask_in, adj_mask_loop, mask, W_gate_in, b_gate_in, W_gate_self)
    wcat, ball = prep_shared(W_in, b_in, W_self)
    in_maps = [
        prep_core_inputs(c, rep, idx, lab, w_in, w_self, wcat, ball, lab_bias=lab_bias)
        for c in range(NCORES)
    ]

    nc = get_nc(lab_bias)
    res = bass_utils.run_bass_kernel_spmd(nc, in_maps, core_ids=list(range(NCORES)))
    out = np.concatenate([unshard_out(r["out"]) for r in res.results], axis=0)
    return out


# revision 8
# speedup vs baseline: 1.3335x; 1.0075x over previous
"""GCNN message-passing layer on 8 Trainium2 NeuronCores (Bass/Tile).

Math (per token m, all within one sentence of L=64 tokens):
    in_pot[m]  = (rep @ W_in)[head(m)] + b_in[lab(m)]
    in_gate[m] = (rep @ W_gate_in)[head(m)] + b_gate_in[lab(m)]
    self_pot   = rep @ W_self ; self_gate = rep @ W_gate_self
    w_d = sigmoid(gate_d) * msoft_d^2
    out = relu(in_pot*w_in + self_pot*w_self) * mask

Sharding: data-parallel over BNK (160 sentences / core). All gathers stay
within a sentence, so shards are independent; weights are replicated.

Device strategy per 128-token tile (2 sentences):
  - The gate paths (rep @ W_gate_*, 0.2% of the FLOPs) run on the host;
    sigmoid(gate)*msoft^2*mask folds into the one-hot scatter values (w_in
    side) and into a per-token aux vector (w_self side). The device never
    computes gates, sigmoids, or masks.
  - rep arrives host-pretransposed (fp16) so DIN sits on partitions. One
    512-column moving operand [W_in | W_self] turns the two projections
    into 4 accumulating matmuls per tile (512-cycle streams fully hide
    each LDWEIGHTS).
  - The within-tile head gather is a matmul with a host-built one-hot
    scatter matrix whose nonzeros carry w_in; it is software-pipelined one
    tile behind the projections so the in-order tensor queue never waits
    on the PSUM->fp16 cast feeding it. (A relation-bias matmul joins the
    same accumulation only when b_in != 0; setup_inputs has b_in == 0.)
  - Tail per tile: one ACT op casts the whole [in_pot|self_pot] PSUM bank
    to fp16 (gather src + self operand), then one fused custom-DVE op
    emits relu(in_pot_gathered + w_self*self_pot) straight to fp16.
  - Output stays partition-major in DRAM ([128, ntiles, dout]) so the DMA
    moves 4KB-contiguous runs; the host de-interleaves.
  - Startup: ~32 throwaway matmuls release the PE HAM clock gate while
    the first DMAs land; wcat rides the Scalar HWDGE queue concurrently
    with rep on the SP queue, and the first four tiles' rep arrives as
    single-tile DMAs, so real matmuls start ~9 us in, still warm.
  - Outputs ride the GpSimd SWDGE queue except the last batch, which goes
    on the Scalar HWDGE queue so the end-of-kernel SWDGE drain finds an
    idle queue.
"""

import numpy as np

import concourse.bass as bass
import concourse.dve_ops as dve_ops
import concourse.mybir as mybir
import concourse.tile as tile
from concourse import bacc, bass_utils
from concourse.dve_spec import C0, C1, Spec, Src0, Src1, lower as dve_lower, relu as dve_relu
from concourse.dve_uop import DveOpSpec


def _register_gated_relu_op():
    """Register a fused custom-DVE op: out = relu(in0*s0 + in1*s1)."""
    name = "GCNN_GATED_RELU_ANT"
    for op in dve_ops.OPS:
        if op.name == name:
            return op
    spec = Spec(
        body=dve_relu(Src0 * C0 + Src1 * C1),
        reference=lambda in0, in1, s0, s1, imm2: np.maximum(
            np.nan_to_num(in0.astype(np.float32) * s0 + in1 * s1,
                          nan=0.0, posinf=np.inf, neginf=-np.inf), 0.0),
    )
    row = dve_ops._CUSTOM_DVE_ROW_BASE + len(dve_ops.OPS)
    dve_ops._SUB_OPCODE_FOR_NAME[name] = row
    shas = {}
    for ver in ("v3", "v4"):
        uops = dve_lower(spec, ver=ver)
        shas[ver] = DveOpSpec(name=name, opcode=row, uops=uops, rd1_en=True).sha(ver)
    op = dve_ops.DveOp(name, spec, subdim=False, uops_sha=shas)
    dve_ops.OPS.append(op)
    dve_ops.CUSTOM_DVE_SPECS[name] = spec
    return op


GATED_RELU = _register_gated_relu_op()

BNK, L, DIN, DOUT, NREL = 1280, 64, 512, 256, 40
NCORES = 8
SPC = BNK // NCORES          # sentences per core
TOK = SPC * L                # tokens per core (10240)
TILE_T = 128                 # tokens per device tile
KC = DIN // 128              # K chunks (4)
NTILES = TOK // TILE_T       # 80
OGROUP = 4                   # tiles per output DMA batch
NWARM = 32                   # HAM warmup matmuls

F32 = mybir.dt.float32
F16 = mybir.dt.float16
NP_MM = np.float16
AF = mybir.ActivationFunctionType


def _in_groups(ntiles):
    """Input DMA batching: single tiles first (fast start), then fours."""
    gs = [(0, 1), (1, 1), (2, 1), (3, 1)]
    i = 4
    while i < ntiles:
        sz = min(4, ntiles - i)
        gs.append((i, sz))
        i += sz
    return gs


def build_nc(ntiles: int = NTILES, lab_bias: bool = False):
    """Build the per-core Bass program (same program on all cores).

    lab_bias=True adds the relation-bias matmul (needed only when b_in is
    nonzero; setup_inputs always produces b_in == 0).
    """
    assert ntiles % OGROUP == 0
    nc = bacc.Bacc("TRN2", target_bir_lowering=False, debug=False)

    # --- DRAM I/O (flat, partition-major; sliced per DMA batch) ---------
    repT_d = nc.dram_tensor("repT", [128, ntiles, KC, TILE_T], F16, kind="ExternalInput")
    scatW_d = nc.dram_tensor("scatW", [TILE_T, ntiles, TILE_T], F16, kind="ExternalInput")
    if lab_bias:
        scatL_d = nc.dram_tensor("scatL", [NREL, ntiles, TILE_T], F16, kind="ExternalInput")
        ball_d = nc.dram_tensor("ball", [NREL, DOUT], F16, kind="ExternalInput")
    wcat_d = nc.dram_tensor("wcat", [128, KC, 2 * DOUT], F16, kind="ExternalInput")
    # aux[:, i] = w_self for tile i; last column is the constant 1.0
    aux_d = nc.dram_tensor("aux", [128, ntiles + 1], F32, kind="ExternalInput")
    # partition-major output: [p, tile, dout]; host de-interleaves
    out_d = nc.dram_tensor("out", [TILE_T, ntiles, DOUT], F16, kind="ExternalOutput")

    groups = _in_groups(ntiles)
    with tile.TileContext(nc) as tc:
        with (
            tc.tile_pool(name="const", bufs=1) as const_pool,
            tc.tile_pool(name="rep", bufs=3) as rep_pool,
            tc.tile_pool(name="scat", bufs=3) as scat_pool,
            tc.tile_pool(name="src", bufs=3) as src_pool,
            tc.tile_pool(name="out", bufs=3) as out_pool,
            tc.tile_pool(name="psum", bufs=3, space="PSUM") as psum_pool,
            tc.tile_pool(name="psum2", bufs=3, space="PSUM") as psum2_pool,
            tc.tile_pool(name="psumw", bufs=1, space="PSUM") as psumw_pool,
        ):
            # --- PE warmup: release the HAM clock gate while DMAs land ---
            wz = const_pool.tile([128, 16], F16)
            nc.gpsimd.memset(wz[:], 0.0)
            wp = psumw_pool.tile([16, 16], F32, tag="warm")
            for _ in range(NWARM):
                nc.tensor.matmul(wp[:], wz[:, 0:16], wz[:], start=True, stop=True)

            # wcat on the Scalar HWDGE queue, concurrent with rep on SP
            wcat_sb = const_pool.tile([128, KC, 2 * DOUT], F16)
            nc.scalar.dma_start(wcat_sb[:], wcat_d[:])
            aux_sb = const_pool.tile([128, ntiles + 1], F32)
            ones = aux_sb[:, ntiles:ntiles + 1]
            ball_sb = const_pool.tile([NREL, DOUT], F16) if lab_bias else None

            pend = None          # (i, src, o_sb, oslot, scat_sb, sslot, scatl_sb)
            pend_out = None      # (ostart, o_sb) awaiting its batched output DMA

            def flush_tail():
                nonlocal pend, pend_out
                if pend is None:
                    return
                i, src, o_sb, oslot, scat_sb, sslot, scatl_sb = pend
                psum_g = psum2_pool.tile([128, DOUT], F32, tag="pg")
                nc.tensor.matmul(psum_g[:], scat_sb[:, sslot, :], src[:, 0:DOUT],
                                 start=True, stop=not lab_bias)
                if lab_bias:
                    nc.tensor.matmul(psum_g[:], scatl_sb[:, sslot, :], ball_sb[:],
                                     start=False, stop=True)
                nc.vector._custom_dve(GATED_RELU, out=o_sb[:, oslot, :],
                                      in0=psum_g[:], in1=src[:, DOUT:2 * DOUT],
                                      s0=ones, s1=aux_sb[:, i:i + 1])
                pend = None
                if oslot == OGROUP - 1:
                    ostart, osb = pend_out
                    eng = nc.scalar if ostart + OGROUP >= ntiles else nc.gpsimd
                    eng.dma_start(out_d[:, ostart:ostart + OGROUP, :], osb[:])
                    pend_out = None

            o_sb = None
            for gi, (i0, sz) in enumerate(groups):
                rep_sb = rep_pool.tile([128, sz, KC, TILE_T], F16, tag="rep")
                nc.sync.dma_start(rep_sb[:], repT_d[:, i0:i0 + sz, :, :])
                if i0 == 0:
                    # batch the first four tiles' scatter in one DMA
                    scat_sb = scat_pool.tile([TILE_T, 4, TILE_T], F16, tag="scath")
                    nc.sync.dma_start(scat_sb[:], scatW_d[:, 0:4, :])
                    s0 = 0
                    if lab_bias:
                        scatl_sb = scat_pool.tile([NREL, 4, TILE_T], F16, tag="scatl")
                        nc.sync.dma_start(scatl_sb[:], scatL_d[:, 0:4, :])
                elif i0 == 1:
                    # must precede flush_tail(tile 0), which reads aux_sb
                    nc.sync.dma_start(aux_sb[:], aux_d[:])
                    if lab_bias:
                        nc.sync.dma_start(ball_sb[:], ball_d[:])
                elif i0 >= 4:
                    scat_sb = scat_pool.tile([TILE_T, sz, TILE_T], F16, tag="scath")
                    nc.sync.dma_start(scat_sb[:], scatW_d[:, i0:i0 + sz, :])
                    s0 = i0
                    if lab_bias:
                        scatl_sb = scat_pool.tile([NREL, sz, TILE_T], F16, tag="scatl")
                        nc.sync.dma_start(scatl_sb[:], scatL_d[:, i0:i0 + sz, :])

                for ti in range(sz):
                    i = i0 + ti
                    if i % OGROUP == 0:
                        o_sb = out_pool.tile([128, OGROUP, DOUT], F16)
                    # [in_pot | self_pot] in one PSUM bank via a fused
                    # 512-column moving operand
                    psum_ab = psum_pool.tile([128, 2 * DOUT], F32, tag="pab")
                    for kc in range(KC):
                        nc.tensor.matmul(psum_ab[:], rep_sb[:, ti, kc, :], wcat_sb[:, kc, :],
                                         start=kc == 0, stop=kc == KC - 1)
                    src = src_pool.tile([128, 2 * DOUT], F16)
                    nc.scalar.activation(src[:], psum_ab[:], AF.Copy)
                    flush_tail()
                    if i % OGROUP == OGROUP - 1:
                        pend_out = (i - OGROUP + 1, o_sb)
                    pend = (i, src, o_sb, i % OGROUP, scat_sb, i - s0,
                            scatl_sb if lab_bias else None)
            flush_tail()

    nc.compile()
    return nc


def _sigmoid(x):
    out = np.empty_like(x, dtype=np.float32)
    pos = x >= 0
    out[pos] = 1.0 / (1.0 + np.exp(-x[pos]))
    ex = np.exp(x[~pos])
    out[~pos] = ex / (1.0 + ex)
    return out


def prep_gates(rep_flat, adj_arc, adj_lab, adj_mask_in, adj_mask_loop, mask,
               W_gate_in, b_gate_in, W_gate_self):
    """Host gate path: per-token gate weights with masks folded in."""
    idx = (adj_arc[..., 0].reshape(-1) * L + adj_arc[..., 1].reshape(-1)).astype(np.int64)
    lab = adj_lab.reshape(-1).astype(np.int64)
    g_in = (rep_flat @ np.asarray(W_gate_in, np.float32)[:, 0])[idx] \
        + np.asarray(b_gate_in, np.float32)[lab, 0]
    g_self = rep_flat @ np.asarray(W_gate_self, np.float32)[:, 0]
    m = np.asarray(mask, np.float32).reshape(-1)
    w_in = _sigmoid(g_in) * np.asarray(adj_mask_in, np.float32).reshape(-1) ** 2 * m
    w_self = _sigmoid(g_self) * np.asarray(adj_mask_loop, np.float32).reshape(-1) ** 2 * m
    return idx, lab, w_in, w_self


def prep_core_inputs(c, rep, idx, lab, w_in, w_self, wcat, ball,
                     ntiles: int = NTILES, lab_bias: bool = False):
    """Build the per-core in_map (host-side shard + layout prep)."""
    tok = ntiles * TILE_T
    lo = c * SPC * L
    rep_s = np.ascontiguousarray(rep[c * SPC:(c + 1) * SPC]).reshape(SPC * L, DIN)[:tok]
    x = rep_s.reshape(ntiles, TILE_T, KC, 128)              # [i, t, kc, k]
    repT = np.ascontiguousarray(x.transpose(3, 0, 2, 1).astype(NP_MM))  # [k, i, kc, t]

    idx_local = idx[lo:lo + tok] - lo
    t_all = np.arange(tok)
    if idx_local.min() < 0 or idx_local.max() >= tok or np.any(idx_local // TILE_T != t_all // TILE_T):
        raise ValueError("head gather escapes its 128-token tile; unsupported input structure")

    w_in_s = w_in[lo:lo + tok].astype(NP_MM)
    scatW = np.zeros((TILE_T, ntiles, TILE_T), NP_MM)
    scatW[idx_local % TILE_T, t_all // TILE_T, t_all % TILE_T] = w_in_s

    aux = np.empty((128, ntiles + 1), np.float32)
    aux[:, :ntiles] = w_self[lo:lo + tok].reshape(ntiles, TILE_T).T
    aux[:, ntiles] = 1.0

    in_map = {"repT": repT, "scatW": scatW, "wcat": wcat, "aux": aux}
    if lab_bias:
        lab_s = lab[lo:lo + tok]
        scatL = np.zeros((NREL, ntiles, TILE_T), NP_MM)
        scatL[lab_s, t_all // TILE_T, t_all % TILE_T] = w_in_s
        in_map["scatL"] = scatL
        in_map["ball"] = ball
    return in_map


def prep_shared(W_in, b_in, W_self):
    wcat = np.concatenate([np.asarray(W_in, np.float32),
                           np.asarray(W_self, np.float32)], axis=1)
    wcat = np.ascontiguousarray(
        wcat.reshape(KC, 128, 2 * DOUT).transpose(1, 0, 2).astype(NP_MM))
    ball = np.ascontiguousarray(np.asarray(b_in, np.float32).astype(NP_MM))
    return wcat, ball


def unshard_out(raw):
    """[128, ntiles, DOUT] fp16 partition-major -> [SPC, L, DOUT] fp32."""
    return raw.transpose(1, 0, 2).astype(np.float32).reshape(SPC, L, DOUT)


_NC_CACHE = {}


def get_nc(lab_bias: bool):
    if lab_bias not in _NC_CACHE:
        _NC_CACHE[lab_bias] = build_nc(lab_bias=lab_bias)
    return _NC_CACHE[lab_bias]


def kernel(rep, adj_mask_in, adj_mask_loop, mask, W_in, b_in, W_gate_in,
           b_gate_in, W_self, W_gate_self, adj_arc_in, adj_lab_in):
    rep = np.asarray(rep, dtype=np.float32)
    b_in = np.asarray(b_in, dtype=np.float32)
    lab_bias = bool(np.any(b_in != 0.0))
    rep_flat = rep.reshape(BNK * L, DIN)
    idx, lab, w_in, w_self = prep_gates(
        rep_flat, np.asarray(adj_arc_in), np.asarray(adj_lab_in),
        adj_mask_in, adj_mask_loop, mask, W_gate_in, b_gate_in, W_gate_self)
    wcat, ball = prep_shared(W_in, b_in, W_self)
    in_maps = [
        prep_core_inputs(c, rep, idx, lab, w_in, w_self, wcat, ball, lab_bias=lab_bias)
        for c in range(NCORES)
    ]

    nc = get_nc(lab_bias)
    res = bass_utils.run_bass_kernel_spmd(nc, in_maps, core_ids=list(range(NCORES)))
    out = np.concatenate([unshard_out(r["out"]) for r in res.results], axis=0)
    return out


# revision 13
# speedup vs baseline: 1.3339x; 1.0003x over previous
"""GCNN message-passing layer on 8 Trainium2 NeuronCores (Bass/Tile).

Math (per token m, all within one sentence of L=64 tokens):
    in_pot[m]  = (rep @ W_in)[head(m)] + b_in[lab(m)]
    in_gate[m] = (rep @ W_gate_in)[head(m)] + b_gate_in[lab(m)]
    self_pot   = rep @ W_self ; self_gate = rep @ W_gate_self
    w_d = sigmoid(gate_d) * msoft_d^2
    out = relu(in_pot*w_in + self_pot*w_self) * mask

Sharding: data-parallel over BNK (160 sentences / core). All gathers stay
within a sentence, so shards are independent; weights are replicated.

Device strategy per 128-token tile (2 sentences):
  - The gate paths (rep @ W_gate_*, 0.2% of the FLOPs) run on the host;
    sigmoid(gate)*msoft^2*mask folds into the one-hot scatter values (w_in
    side) and into a per-token aux vector (w_self side). The device never
    computes gates, sigmoids, or masks.
  - rep arrives host-pretransposed (fp16) so DIN sits on partitions. One
    512-column moving operand [W_in | W_self] turns the two projections
    into 4 accumulating matmuls per tile (512-cycle streams fully hide
    each LDWEIGHTS).
  - The within-tile head gather is a matmul with a host-built one-hot
    scatter matrix whose nonzeros carry w_in; it is software-pipelined one
    tile behind the projections so the in-order tensor queue never waits
    on the PSUM->fp16 cast feeding it. (A relation-bias matmul joins the
    same accumulation only when b_in != 0; setup_inputs has b_in == 0.)
  - Tail per tile: one ACT op casts the whole [in_pot|self_pot] PSUM bank
    to fp16 (gather src + self operand), then one fused custom-DVE op
    emits relu(in_pot_gathered + w_self*self_pot) straight to fp16.
  - Output stays partition-major in DRAM ([128, ntiles, dout]) so the DMA
    moves 4KB-contiguous runs; the host de-interleaves.
  - Startup: ~32 throwaway matmuls release the PE HAM clock gate while
    the first DMAs land; wcat rides the Scalar HWDGE queue concurrently
    with rep on the SP queue, and the first four tiles' rep arrives as
    single-tile DMAs, so real matmuls start ~9 us in, still warm.
  - Outputs ride the GpSimd SWDGE queue except the last batch, which goes
    on the Scalar HWDGE queue so the end-of-kernel SWDGE drain finds an
    idle queue.
"""

import numpy as np

import concourse.bass as bass
import concourse.dve_ops as dve_ops
import concourse.mybir as mybir
import concourse.tile as tile
from concourse import bacc, bass_utils
from concourse.dve_spec import C0, C1, Spec, Src0, Src1, lower as dve_lower, relu as dve_relu
from concourse.dve_uop import DveOpSpec


def _register_gated_relu_op():
    """Register a fused custom-DVE op: out = relu(in0*s0 + in1*s1)."""
    name = "GCNN_GATED_RELU_ANT"
    for op in dve_ops.OPS:
        if op.name == name:
            return op
    spec = Spec(
        body=dve_relu(Src0 * C0 + Src1 * C1),
        reference=lambda in0, in1, s0, s1, imm2: np.maximum(
            np.nan_to_num(in0.astype(np.float32) * s0 + in1 * s1,
                          nan=0.0, posinf=np.inf, neginf=-np.inf), 0.0),
    )
    row = dve_ops._CUSTOM_DVE_ROW_BASE + len(dve_ops.OPS)
    dve_ops._SUB_OPCODE_FOR_NAME[name] = row
    shas = {}
    for ver in ("v3", "v4"):
        uops = dve_lower(spec, ver=ver)
        shas[ver] = DveOpSpec(name=name, opcode=row, uops=uops, rd1_en=True).sha(ver)
    op = dve_ops.DveOp(name, spec, subdim=False, uops_sha=shas)
    dve_ops.OPS.append(op)
    dve_ops.CUSTOM_DVE_SPECS[name] = spec
    return op


GATED_RELU = _register_gated_relu_op()

BNK, L, DIN, DOUT, NREL = 1280, 64, 512, 256, 40
NCORES = 8
SPC = BNK // NCORES          # sentences per core
TOK = SPC * L                # tokens per core (10240)
TILE_T = 128                 # tokens per device tile
KC = DIN // 128              # K chunks (4)
NTILES = TOK // TILE_T       # 80
OGROUP = 4                   # tiles per output DMA batch
NWARM = 32                   # HAM warmup matmuls

F32 = mybir.dt.float32
F16 = mybir.dt.float16
NP_MM = np.float16
AF = mybir.ActivationFunctionType


def _in_groups(ntiles):
    """Input DMA batching: single tiles first (fast start), then fours."""
    gs = [(0, 1), (1, 1), (2, 1), (3, 1)]
    i = 4
    while i < ntiles:
        sz = min(4, ntiles - i)
        gs.append((i, sz))
        i += sz
    return gs


def build_nc(ntiles: int = NTILES, lab_bias: bool = False):
    """Build the per-core Bass program (same program on all cores).

    lab_bias=True adds the relation-bias matmul (needed only when b_in is
    nonzero; setup_inputs always produces b_in == 0).
    """
    assert ntiles % OGROUP == 0
    nc = bacc.Bacc("TRN2", target_bir_lowering=False, debug=False)

    # --- DRAM I/O (flat, partition-major; sliced per DMA batch) ---------
    repT_d = nc.dram_tensor("repT", [128, ntiles, KC, TILE_T], F16, kind="ExternalInput")
    scatW_d = nc.dram_tensor("scatW", [TILE_T, ntiles, TILE_T], F16, kind="ExternalInput")
    if lab_bias:
        scatL_d = nc.dram_tensor("scatL", [NREL, ntiles, TILE_T], F16, kind="ExternalInput")
        ball_d = nc.dram_tensor("ball", [NREL, DOUT], F16, kind="ExternalInput")
    wcat_d = nc.dram_tensor("wcat", [128, KC, 2 * DOUT], F16, kind="ExternalInput")
    # aux[:, i] = w_self for tile i; last column is the constant 1.0
    aux_d = nc.dram_tensor("aux", [128, ntiles + 1], F32, kind="ExternalInput")
    # partition-major output: [p, tile, dout]; host de-interleaves
    out_d = nc.dram_tensor("out", [TILE_T, ntiles, DOUT], F16, kind="ExternalOutput")

    groups = _in_groups(ntiles)
    with tile.TileContext(nc) as tc:
        with (
            tc.tile_pool(name="const", bufs=1) as const_pool,
            tc.tile_pool(name="rep", bufs=6) as rep_pool,
            tc.tile_pool(name="scat", bufs=5) as scat_pool,
            tc.tile_pool(name="src", bufs=3) as src_pool,
            tc.tile_pool(name="out", bufs=3) as out_pool,
            tc.tile_pool(name="psum", bufs=3, space="PSUM") as psum_pool,
            tc.tile_pool(name="psum2", bufs=3, space="PSUM") as psum2_pool,
            tc.tile_pool(name="psumw", bufs=1, space="PSUM") as psumw_pool,
        ):
            # --- PE warmup: release the HAM clock gate while DMAs land ---
            wz = const_pool.tile([128, 16], F16)
            nc.gpsimd.memset(wz[:], 0.0)
            wp = psumw_pool.tile([16, 16], F32, tag="warm")
            for _ in range(NWARM):
                nc.tensor.matmul(wp[:], wz[:, 0:16], wz[:], start=True, stop=True)

            # wcat on the Scalar HWDGE queue, concurrent with rep on SP;
            # per-kc slices so the first matmul waits on 128KB, not 512KB
            wcat_sb = [const_pool.tile([128, 2 * DOUT], F16, tag=f"wcat{kc}",
                                       name=f"wcat{kc}")
                       for kc in range(KC)]
            for kc in range(KC):
                nc.scalar.dma_start(wcat_sb[kc][:], wcat_d[:, kc, :])
            aux_sb = const_pool.tile([128, ntiles + 1], F32)
            ones = aux_sb[:, ntiles:ntiles + 1]
            ball_sb = const_pool.tile([NREL, DOUT], F16) if lab_bias else None

            pend = None          # (i, src, o_sb, oslot, scat_sb, sslot, scatl_sb)
            pend_out = None      # (ostart, o_sb) awaiting its batched output DMA

            def flush_tail():
                nonlocal pend, pend_out
                if pend is None:
                    return
                i, src, o_sb, oslot, scat_sb, sslot, scatl_sb = pend
                psum_g = psum2_pool.tile([128, DOUT], F32, tag="pg")
                nc.tensor.matmul(psum_g[:], scat_sb[:, sslot, :], src[:, 0:DOUT],
                                 start=True, stop=not lab_bias)
                if lab_bias:
                    nc.tensor.matmul(psum_g[:], scatl_sb[:, sslot, :], ball_sb[:],
                                     start=False, stop=True)
                nc.vector._custom_dve(GATED_RELU, out=o_sb[:, oslot, :],
                                      in0=psum_g[:], in1=src[:, DOUT:2 * DOUT],
                                      s0=ones, s1=aux_sb[:, i:i + 1])
                pend = None
                if i >= ntiles - OGROUP:
                    # drain the final tiles one by one on the HWDGE queue so
                    # the last transfer is 64KB, not 256KB
                    nc.scalar.dma_start(out_d[:, i:i + 1, :], o_sb[:, oslot:oslot + 1, :])
                    pend_out = None
                elif oslot == OGROUP - 1:
                    ostart, osb = pend_out
                    nc.gpsimd.dma_start(out_d[:, ostart:ostart + OGROUP, :], osb[:])
                    pend_out = None

            o_sb = None
            for gi, (i0, sz) in enumerate(groups):
                rep_sb = rep_pool.tile([128, sz, KC, TILE_T], F16, tag="rep")
                nc.sync.dma_start(rep_sb[:], repT_d[:, i0:i0 + sz, :, :])
                if i0 == 0:
                    # batch the first four tiles' scatter in one DMA
                    scat_sb = scat_pool.tile([TILE_T, 4, TILE_T], F16, tag="scath")
                    nc.sync.dma_start(scat_sb[:], scatW_d[:, 0:4, :])
                    s0 = 0
                    if lab_bias:
                        scatl_sb = scat_pool.tile([NREL, 4, TILE_T], F16, tag="scatl")
                        nc.sync.dma_start(scatl_sb[:], scatL_d[:, 0:4, :])
                elif i0 == 1:
                    # must precede flush_tail(tile 0), which reads aux_sb
                    nc.sync.dma_start(aux_sb[:], aux_d[:])
                    if lab_bias:
                        nc.sync.dma_start(ball_sb[:], ball_d[:])
                elif i0 >= 4:
                    scat_sb = scat_pool.tile([TILE_T, sz, TILE_T], F16, tag="scath")
                    nc.sync.dma_start(scat_sb[:], scatW_d[:, i0:i0 + sz, :])
                    s0 = i0
                    if lab_bias:
                        scatl_sb = scat_pool.tile([NREL, sz, TILE_T], F16, tag="scatl")
                        nc.sync.dma_start(scatl_sb[:], scatL_d[:, i0:i0 + sz, :])

                for ti in range(sz):
                    i = i0 + ti
                    if i % OGROUP == 0:
                        o_sb = out_pool.tile([128, OGROUP, DOUT], F16)
                    # [in_pot | self_pot] in one PSUM bank via a fused
                    # 512-column moving operand
                    psum_ab = psum_pool.tile([128, 2 * DOUT], F32, tag="pab")
                    for kc in range(KC):
                        nc.tensor.matmul(psum_ab[:], rep_sb[:, ti, kc, :], wcat_sb[kc][:],
                                         start=kc == 0, stop=kc == KC - 1)
                    src = src_pool.tile([128, 2 * DOUT], F16)
                    nc.scalar.activation(src[:], psum_ab[:], AF.Copy)
                    flush_tail()
                    if i % OGROUP == OGROUP - 1:
                        pend_out = (i - OGROUP + 1, o_sb)
                    pend = (i, src, o_sb, i % OGROUP, scat_sb, i - s0,
                            scatl_sb if lab_bias else None)
            flush_tail()

    nc.compile()
    return nc


def _sigmoid(x):
    out = np.empty_like(x, dtype=np.float32)
    pos = x >= 0
    out[pos] = 1.0 / (1.0 + np.exp(-x[pos]))
    ex = np.exp(x[~pos])
    out[~pos] = ex / (1.0 + ex)
    return out


def prep_gates(rep_flat, adj_arc, adj_lab, adj_mask_in, adj_mask_loop, mask,
               W_gate_in, b_gate_in, W_gate_self):
    """Host gate path: per-token gate weights with masks folded in."""
    idx = (adj_arc[..., 0].reshape(-1) * L + adj_arc[..., 1].reshape(-1)).astype(np.int64)
    lab = adj_lab.reshape(-1).astype(np.int64)
    g_in = (rep_flat @ np.asarray(W_gate_in, np.float32)[:, 0])[idx] \
        + np.asarray(b_gate_in, np.float32)[lab, 0]
    g_self = rep_flat @ np.asarray(W_gate_self, np.float32)[:, 0]
    m = np.asarray(mask, np.float32).reshape(-1)
    w_in = _sigmoid(g_in) * np.asarray(adj_mask_in, np.float32).reshape(-1) ** 2 * m
    w_self = _sigmoid(g_self) * np.asarray(adj_mask_loop, np.float32).reshape(-1) ** 2 * m
    return idx, lab, w_in, w_self


def prep_core_inputs(c, rep, idx, lab, w_in, w_self, wcat, ball,
                     ntiles: int = NTILES, lab_bias: bool = False):
    """Build the per-core in_map (host-side shard + layout prep)."""
    tok = ntiles * TILE_T
    lo = c * SPC * L
    rep_s = np.ascontiguousarray(rep[c * SPC:(c + 1) * SPC]).reshape(SPC * L, DIN)[:tok]
    x = rep_s.reshape(ntiles, TILE_T, KC, 128)              # [i, t, kc, k]
    repT = np.ascontiguousarray(x.transpose(3, 0, 2, 1).astype(NP_MM))  # [k, i, kc, t]

    idx_local = idx[lo:lo + tok] - lo
    t_all = np.arange(tok)
    if idx_local.min() < 0 or idx_local.max() >= tok or np.any(idx_local // TILE_T != t_all // TILE_T):
        raise ValueError("head gather escapes its 128-token tile; unsupported input structure")

    w_in_s = w_in[lo:lo + tok].astype(NP_MM)
    scatW = np.zeros((TILE_T, ntiles, TILE_T), NP_MM)
    scatW[idx_local % TILE_T, t_all // TILE_T, t_all % TILE_T] = w_in_s

    aux = np.empty((128, ntiles + 1), np.float32)
    aux[:, :ntiles] = w_self[lo:lo + tok].reshape(ntiles, TILE_T).T
    aux[:, ntiles] = 1.0

    in_map = {"repT": repT, "scatW": scatW, "wcat": wcat, "aux": aux}
    if lab_bias:
        lab_s = lab[lo:lo + tok]
        scatL = np.zeros((NREL, ntiles, TILE_T), NP_MM)
        scatL[lab_s, t_all // TILE_T, t_all % TILE_T] = w_in_s
        in_map["scatL"] = scatL
        in_map["ball"] = ball
    return in_map


def prep_shared(W_in, b_in, W_self):
    wcat = np.concatenate([np.asarray(W_in, np.float32),
                           np.asarray(W_self, np.float32)], axis=1)
    wcat = np.ascontiguousarray(
        wcat.reshape(KC, 128, 2 * DOUT).transpose(1, 0, 2).astype(NP_MM))
    ball = np.ascontiguousarray(np.asarray(b_in, np.float32).astype(NP_MM))
    return wcat, ball


def unshard_out(raw):
    """[128, ntiles, DOUT] fp16 partition-major -> [SPC, L, DOUT] fp32."""
    return raw.transpose(1, 0, 2).astype(np.float32).reshape(SPC, L, DOUT)


_NC_CACHE = {}


def get_nc(lab_bias: bool):
    if lab_bias not in _NC_CACHE:
        _NC_CACHE[lab_bias] = build_nc(lab_bias=lab_bias)
    return _NC_CACHE[lab_bias]


def kernel(rep, adj_mask_in, adj_mask_loop, mask, W_in, b_in, W_gate_in,
           b_gate_in, W_self, W_gate_self, adj_arc_in, adj_lab_in):
    rep = np.asarray(rep, dtype=np.float32)
    b_in = np.asarray(b_in, dtype=np.float32)
    lab_bias = bool(np.any(b_in != 0.0))
    rep_flat = rep.reshape(BNK * L, DIN)
    idx, lab, w_in, w_self = prep_gates(
        rep_flat, np.asarray(adj_arc_in), np.asarray(adj_lab_in),
        adj_mask_in, adj_mask_loop, mask, W_gate_in, b_gate_in, W_gate_self)
    wcat, ball = prep_shared(W_in, b_in, W_self)
    in_maps = [
        prep_core_inputs(c, rep, idx, lab, w_in, w_self, wcat, ball, lab_bias=lab_bias)
        for c in range(NCORES)
    ]

    nc = get_nc(lab_bias)
    res = bass_utils.run_bass_kernel_spmd(nc, in_maps, core_ids=list(range(NCORES)))
    out = np.concatenate([unshard_out(r["out"]) for r in res.results], axis=0)
    return out
